# revision 9
# baseline (speedup 1.0000x reference)
"""Trainium2 Bass kernel for: 3x3 conv (reflect pad) + BatchNorm + LeakyReLU + mask.

Input  x:    (1, 64, 512, 512) f32
       W:    (128, 64, 3, 3)   f32
       gamma/beta/mean/var: (128,) f32
       mask: (1, 128, 512, 512) int32 (0/1)
Output (1, 128, 512, 512) f32

Strategy (8 cores, SPMD):
  - Shard H spatially: core c computes output rows [64c, 64c+64).
  - Even/odd row deinterleave (x shipped ONCE, no duplication):
      partition p<64   : channel p, even padded slab rows (2q)   at offset q*WP
      partition 64+p   : channel p, odd  padded slab rows (2q+1) at offset q*WP
    For even output row y=2q, taps dy=0 (row 2q) and dy=1 (row 2q+1) live on
    complementary partition halves at the SAME offset q -> one K=128 matmul.
    For odd row y=2q+1, taps dy=1 (row 2q+2) and dy=2 (row 2q+3) pair at
    offset q+1 -> one K=128 matmul. The leftover taps (dy=2 of the even row,
    on partitions 0..63, and dy=0 of the odd row, on partitions 64..127) are
    K=64 matmuls on disjoint PE row halves issued back-to-back so they
    execute concurrently. 9 matmul slots per row pair = MAC-optimal.
  - Four row-pairs are batched per iteration (24 K=128 MMs, then 12 dual
    K=64 slots, all 8 PSUM banks): the PE loses ~370 ns per K128<->K64
    row-config switch cycle, so batching quarters that cost. PSUM banks
    stop mid-K64-block and ACT drains them during the next K128 block.
  - All 9 stationary weight tiles ship as ONE [128, 1152] wall (a dma_start
    trigger costs ~630 ns on the issuing engine; 10 separate loads starved
    the PE at startup). The wall rides the scalar HWDGE ring (the gpsimd
    SWDGE ring delivered it ~3 us late).
  - Engine/queue split: gpsimd = mask prefetch; sync = x even chunks + JIT
    x blocks + output stores (idle otherwise); scalar = weight wall + bn +
    x odd chunks + ACT Lrelu epilogue; vector = mask multiply. A dummy
    1-col Lrelu right after bn lands pre-warms the ACT table (~1.3 us).
  - PSUM accumulates fp32; epilogue = ACT Lrelu(psum*scale+shift) -> bf16,
    DVE multiply by uint8 mask in place; bf16 stores (host upcasts to f32).
"""

import numpy as np
import ml_dtypes

import concourse.bacc as bacc
import concourse.bass as bass
import concourse.mybir as mybir
import concourse.tile as tile
from concourse.bass_utils import run_bass_kernel_spmd

bf16 = ml_dtypes.bfloat16

N_CORES = 8
C_IN = 64
C_OUT = 128
H = 512
W_IMG = 512
HS = H // N_CORES            # 64 output rows per core
WP = W_IMG + 2               # 514 padded columns
NQ = HS // 2 + 1             # 33 deinterleaved rows per partition half
FREE = NQ * WP               # per-partition free elems of the x image
G = 8                        # output rows per mask group
SG = 4                       # output rows per store tile (= iteration rows)
NSING = 10                   # x chunks preloaded before the loop
LEAK = 0.01
EPS = 1e-5

_CACHE = {}
LAST_RESULTS = None          # BassKernelResults of the last run (for test.py)


def _build_program(hw_lrelu: bool = True) -> bass.Bass:
    """hw_lrelu=True uses the ACT engine's native Lrelu (not implemented in
    CoreSim); False uses an Identity + DVE max(z*a, z) fallback."""
    nc = bacc.Bacc("TRN2", target_bir_lowering=False, debug=False,
                   num_devices=N_CORES)
    f32 = mybir.dt.float32
    bf = mybir.dt.bfloat16
    u8 = mybir.dt.uint8

    xs_d = nc.dram_tensor("xs", [128, FREE], bf, kind="ExternalInput")
    wp_d = nc.dram_tensor("wp", [128, 9 * C_OUT], bf, kind="ExternalInput")
    bn_d = nc.dram_tensor("bn", [C_OUT, 2], f32, kind="ExternalInput")
    mk_d = nc.dram_tensor("msk", [C_OUT, HS * W_IMG], u8, kind="ExternalInput")
    out_d = nc.dram_tensor("out", [C_OUT, HS * W_IMG], bf, kind="ExternalOutput")

    with tile.TileContext(nc) as tc:
        with tc.tile_pool(name="const", bufs=1) as cpool, \
             tc.tile_pool(name="xp", bufs=1) as xpool, \
             tc.tile_pool(name="mp", bufs=3) as mpool, \
             tc.tile_pool(name="zp", bufs=2) as zpool, \
             tc.tile_pool(name="op", bufs=3) as opool, \
             tc.tile_pool(name="ps", bufs=8, space="PSUM") as ppool:

            wall = cpool.tile([128, 9 * C_OUT], bf, name="wall", tag="wall")
            bn = cpool.tile([C_OUT, 2], f32, name="bn_t", tag="bn_t")
            xs = xpool.tile([128, FREE], bf, name="xs_t", tag="xs_t")

            def wcol(j):
                return slice(j * C_OUT, (j + 1) * C_OUT)

            def load_x(q0, q1, eng):
                eng.dma_start(out=xs[:, q0 * WP:q1 * WP],
                              in_=xs_d[:, q0 * WP:q1 * WP])

            # weight wall + bn on the scalar HWDGE ring; early x chunks as
            # three ascending blocks on sync (descriptor processing, not
            # bytes, limits startup DMA — blocks cost the same descriptor
            # count as singles); the rest stream as JIT blocks from the loop
            nc.scalar.dma_start(out=wall[:], in_=wp_d[:])
            load_x(0, 2, nc.sync)
            nc.scalar.dma_start(out=bn[:], in_=bn_d[:])
            load_x(2, 6, nc.sync)
            load_x(6, NSING, nc.sync)
            if hw_lrelu:
                # pre-warm the Lrelu ACT table (lazy load costs ~1.3 us on
                # the first activation otherwise)
                warm = zpool.tile([C_OUT, 1], f32, name="warm", tag="warm")
                nc.scalar.activation(warm[:], bn[:, 0:1],
                                     mybir.ActivationFunctionType.Lrelu,
                                     alpha=LEAK)

            mts = {}

            def load_mask(g):
                mt = mpool.tile([C_OUT, G * W_IMG], u8, name="mt", tag="mt")
                nc.gpsimd.dma_start(
                    out=mt[:], in_=mk_d[:, g * G * W_IMG:(g + 1) * G * W_IMG])
                mts[g] = mt

            ot = None

            def epilogue(y, pst):
                seg = slice((y % SG) * W_IMG, (y % SG + 1) * W_IMG)
                mt = mts[y // G]
                mseg = slice((y % G) * W_IMG, (y % G + 1) * W_IMG)
                if hw_lrelu:
                    nc.scalar.activation(
                        ot[:, seg], pst[:],
                        mybir.ActivationFunctionType.Lrelu,
                        bias=bn[:, 1:2], scale=bn[:, 0:1], alpha=LEAK)
                else:
                    zt = zpool.tile([C_OUT, W_IMG], f32, name="zt", tag="zt")
                    nc.scalar.activation(
                        zt[:], pst[:],
                        mybir.ActivationFunctionType.Identity,
                        bias=bn[:, 1:2], scale=bn[:, 0:1])
                    nc.vector.scalar_tensor_tensor(
                        ot[:, seg], zt[:], LEAK, zt[:],
                        op0=mybir.AluOpType.mult, op1=mybir.AluOpType.max)
                nc.vector.tensor_tensor(ot[:, seg], ot[:, seg], mt[:, mseg],
                                        op=mybir.AluOpType.mult)
                # stores ride the sync HWDGE ring (idle mid-kernel); the last
                # iteration stores in halves so the drain overlaps compute
                last = (y // SG == HS // SG - 1)
                d0 = (y // SG) * SG * W_IMG
                if not last:
                    if y % SG == SG - 1:
                        nc.sync.dma_start(out=out_d[:, d0:d0 + SG * W_IMG],
                                          in_=ot[:])
                elif y % SG == 1:
                    nc.sync.dma_start(out=out_d[:, d0:d0 + 2 * W_IMG],
                                      in_=ot[:, 0:2 * W_IMG])
                elif y % SG == 3:
                    nc.sync.dma_start(
                        out=out_d[:, d0 + 2 * W_IMG:d0 + SG * W_IMG],
                        in_=ot[:, 2 * W_IMG:SG * W_IMG])

            NP = 4                        # row-pairs batched per iteration
            NIT = HS // (2 * NP)          # iterations; 8 rows each
            for k in range(NIT):
                y = 2 * NP * k
                g = y // G
                if g == 0:
                    load_mask(0)
                    load_mask(1)
                elif g + 1 < HS // G:
                    load_mask(g + 1)
                # JIT x blocks: issued on sync behind the previous store
                # trigger, so delivery is paced by compute progress
                if k >= 1:
                    q0 = NSING + 4 * (k - 1)
                    if q0 < NQ:
                        load_x(q0, min(NQ, q0 + 4), nc.sync)
                ps = [ppool.tile([C_OUT, W_IMG], f32, name=f"ps{j}", tag="pst")
                      for j in range(2 * NP)]
                qq = [y // 2 + j for j in range(NP)]
                # K64 leftover block FIRST (start=True): the stops then land
                # in the K128 block at pair granularity (~1.3 us apart from
                # t+3.2 us), giving ACT a ~4 us phase lead on PSUM-bank
                # recycling for the next iteration
                for j in range(NP):
                    for dx in range(3):
                        off_e = (qq[j] + 1) * WP + dx
                        off_o = qq[j] * WP + dx
                        nc.tensor.matmul(ps[2 * j][:], wall[0:64, wcol(6 + dx)],
                                         xs[0:64, off_e:off_e + W_IMG],
                                         start=(dx == 0), stop=False)
                        nc.tensor.matmul(ps[2 * j + 1][:],
                                         wall[64:128, wcol(6 + dx)],
                                         xs[64:128, off_o:off_o + W_IMG],
                                         start=(dx == 0), stop=False)
                for j in range(NP):
                    for dx in range(3):
                        off = qq[j] * WP + dx
                        nc.tensor.matmul(ps[2 * j][:], wall[:, wcol(dx)],
                                         xs[:, off:off + W_IMG],
                                         start=False, stop=(dx == 2))
                    for dx in range(3):
                        off = (qq[j] + 1) * WP + dx
                        nc.tensor.matmul(ps[2 * j + 1][:], wall[:, wcol(3 + dx)],
                                         xs[:, off:off + W_IMG],
                                         start=False, stop=(dx == 2))
                for j in range(2 * NP):
                    yy = y + j
                    if yy % SG == 0:
                        ot = opool.tile([C_OUT, SG * W_IMG], bf,
                                        name="ot", tag="ot")
                    epilogue(yy, ps[j])
    nc.compile()
    return nc


def _get_program(hw_lrelu: bool = True) -> bass.Bass:
    key = ("nc", hw_lrelu)
    if key not in _CACHE:
        _CACHE[key] = _build_program(hw_lrelu)
    return _CACHE[key]


def make_in_maps(x, W, gamma, beta, mean, var, mask):
    """Host-side shard/pack of full inputs into per-core in_maps."""
    x = np.asarray(x, np.float32)
    W = np.asarray(W, np.float32)
    gamma = np.asarray(gamma, np.float32)
    beta = np.asarray(beta, np.float32)
    mean = np.asarray(mean, np.float32)
    var = np.asarray(var, np.float32)
    mask = np.asarray(mask)

    xp = np.pad(x[0], ((0, 0), (1, 1), (1, 1)), mode="reflect")   # [64,514,514]
    xpb = xp.astype(bf16)

    # stationary weight wall [128, 9*C_OUT]; [p, j*128+co] = W[co,ch(p),dy,dx]
    wp = np.zeros((9, 128, C_OUT), np.float32)
    wt = [W[:, :, dy, :].transpose(1, 0, 2) for dy in range(3)]   # [ci,co,dx]
    for dx in range(3):
        wp[dx, 0:64] = wt[0][:, :, dx]        # even rows: dy=0 on even half
        wp[dx, 64:128] = wt[1][:, :, dx]      #            dy=1 on odd half
        wp[3 + dx, 0:64] = wt[1][:, :, dx]    # odd rows:  dy=1 on even half
        wp[3 + dx, 64:128] = wt[2][:, :, dx]  #            dy=2 on odd half
        wp[6 + dx, 0:64] = wt[2][:, :, dx]    # leftovers: even-row dy=2
        wp[6 + dx, 64:128] = wt[0][:, :, dx]  #            odd-row dy=0
    wall = np.ascontiguousarray(
        wp.transpose(1, 0, 2).reshape(128, 9 * C_OUT)).astype(bf16)

    inv = 1.0 / np.sqrt(var + EPS)
    bn = np.stack([gamma * inv, beta - mean * gamma * inv],
                  axis=1).astype(np.float32)                      # [128,2]

    m8 = mask[0].astype(np.uint8)                                 # [128,512,512]

    in_maps = []
    for c in range(N_CORES):
        S = xpb[:, HS * c:HS * c + HS + 2, :]                     # [64,66,514]
        even = np.ascontiguousarray(S[:, 0::2, :]).reshape(C_IN, FREE)
        odd = np.ascontiguousarray(S[:, 1::2, :]).reshape(C_IN, FREE)
        xs_c = np.concatenate([even, odd], axis=0)                # [128, FREE]
        mk_c = np.ascontiguousarray(
            m8[:, HS * c:HS * c + HS, :]).reshape(C_OUT, HS * W_IMG)
        in_maps.append(dict(xs=xs_c, wp=wall, bn=bn, msk=mk_c))
    return in_maps


def kernel(x, W, gamma, beta, mean, var, mask, _trace=False):
    global LAST_RESULTS
    nc = _get_program()
    in_maps = make_in_maps(x, W, gamma, beta, mean, var, mask)
    res = run_bass_kernel_spmd(nc, in_maps, list(range(N_CORES)), trace=_trace)
    LAST_RESULTS = res
    out = np.empty((1, C_OUT, H, W_IMG), np.float32)
    for c in range(N_CORES):
        out[0, :, HS * c:HS * c + HS, :] = \
            np.asarray(res.results[c]["out"]).astype(np.float32) \
              .reshape(C_OUT, HS, W_IMG)
    return out
